# revision 57
# baseline (speedup 1.0000x reference)
"""Trainium2 Bass kernel for MemoryEfficientCrossAttention (seq-parallel v2).

Problem (hardcoded): B=2, Q=2048, K=4096, HIDDEN=1024, HEADS=16, HEAD_DIM=64.
  out = softmax((x_q W_q)(x_k W_k)^T / sqrt(64)) (x_v W_v) W_o

Sharding over 8 NeuronCores: core = b*4 + kq
  b in {0,1}: batch;  kq in {0..3}: K-quarter (1024 contiguous k rows)
Each core projects q for ALL 2048 rows of its batch (4x redundant but cheap),
k/v for its local 1024 k rows (no redundancy), computes unnormalized
attention ctx^T (with a ones-column for the softmax denominators) against its
local k, then a 4-core ReduceScatter per 512-q block sums the partial ctx and
scatters 128-q fragments; each core normalizes its fragments and applies W_o
for its 4x128 output rows.  No other collectives.

Precision: activations and weights are cast to bf16 for the QKV projections
and attention matmuls (psum accumulation stays f32); the xbar DMA-transpose of
x runs on the single bf16 plane.  W_o path stays f32r.  Expected rel err
~5e-3 versus the f32 reference (gate is 2e-2).
"""

import os
import sys
import time

import numpy as np

sys.path.insert(0, "/opt/trn_rl_repo")

import concourse.bass as bass  # noqa: E402
import concourse.mybir as mybir  # noqa: E402
import concourse.tile as tile  # noqa: E402
from concourse import bacc  # noqa: E402

F32 = mybir.dt.float32
F32R = mybir.dt.float32r
BF16 = mybir.dt.bfloat16

HID = 1024
HEADS = 16
HD = 64
B = 2
Q = 2048
KL = 4096
NCORE = 8
KLOC = KL // 4        # local k rows per core
NCH = HID // 128      # 8 hidden chunks
CH_RT = 2             # 128-row tiles per transpose chunk
CHR = CH_RT * 128     # 256 rows per chunk
NKB = KLOC // 128     # 8 local k-blocks
NQB = 4               # 512-q blocks
QB = Q // NQB         # 512
GKB = 2               # k-blocks per score/exp group
SCALE = HD ** -0.5

_CACHED_NC = None


def _build():
    nc = bacc.Bacc("TRN2", target_bir_lowering=False, debug=False,
                   num_devices=NCORE)

    x_q = nc.dram_tensor("x_q", [Q, HID], F32, kind="ExternalInput")
    x_k = nc.dram_tensor("x_k", [KLOC, HID], F32, kind="ExternalInput")
    x_v = nc.dram_tensor("x_v", [KLOC, HID], F32, kind="ExternalInput")
    w_q = nc.dram_tensor("w_q", [HID, HID], F32, kind="ExternalInput")
    w_k = nc.dram_tensor("w_k", [HID, HID], F32, kind="ExternalInput")
    w_v = nc.dram_tensor("w_v", [HID, HID], F32, kind="ExternalInput")
    w_o = nc.dram_tensor("w_o", [HID, HID], F32, kind="ExternalInput")
    out_frag = nc.dram_tensor("out_frag", [NQB, 128, HID], F32,
                              kind="ExternalOutput")

    from contextlib import ExitStack

    with tile.TileContext(nc, pool_alloc_mode="queue") as tc:
        with tc.tile_pool(name="dram", bufs=1, space="DRAM") as dram:
            est = ExitStack()
            pp = est.enter_context(tc.tile_pool(name="persist", bufs=1))
            # partial ctx^T per 512-q block:
            # [head-half, qquad, head%8, d(+denom), 128 q]
            shared = "Shared" if os.environ.get("KSHARED", "0") == "1" \
                else "Local"
            rs_mode = os.environ.get("KRS", "half")
            # seg_h heads per collective segment; each segment is its own
            # contiguous DRAM tensor (BIR requires contiguous collective in).
            seg_h = HEADS if rs_mode in ("full", "a2a") else                 int(os.environ.get("KSEG", "8"))
            nseg = HEADS // seg_h
            ctx_part = [[dram.tile([4, seg_h, HD + 1, 128], BF16,
                                   name=f"ctx_part{i}_{s}")
                         for s in range(nseg)]
                        for i in range(NQB)]
            a2a_out = [dram.tile([4, HEADS, HD + 1, 128], BF16,
                                 name=f"a2a_out{i}")
                       for i in range(NQB)]
            frag = [dram.tile([HEADS, HD + 1, 128], BF16, name=f"frag{i}",
                              addr_space=shared)
                    for i in range(NQB)]

            qTh = pp.tile([128, NCH, Q], BF16)      # strip s = heads 2s,2s+1
            kTh = pp.tile([128, NCH, KLOC], BF16)
            v_aug = pp.tile([128, NKB, HEADS, HD + 1], BF16)

            ones = pp.tile([128, NKB * HEADS], BF16, name="ones")
            nc.vector.memset(ones[:], 1.0)
            nc.vector.tensor_copy(
                v_aug[:, :, :, HD],
                ones[:].rearrange("p (a b) -> p a b", a=NKB))

            # ---------------- staging + projections ----------------
            proj_st = ExitStack()
            xs = proj_st.enter_context(tc.tile_pool(name="xstage", bufs=2))
            xb = proj_st.enter_context(tc.tile_pool(name="xbf", bufs=3))
            xbt = proj_st.enter_context(tc.tile_pool(name="xT", bufs=4))
            ws = proj_st.enter_context(tc.tile_pool(name="wstage", bufs=1))
            wb = proj_st.enter_context(tc.tile_pool(name="wbf", bufs=8))
            # proj psum shares the score-tile ring (stp below): same shape and
            # tag so the 8 banks split 6 (3x2-bank score bufs) + 2 (ctx).
            stp = proj_st.enter_context(
                tc.tile_pool(name="pscore", bufs=3, space="PSUM"))

            def proj_ps(name):
                t = stp.tile([128, GKB, QB], F32, tag="st", name=name)
                return t[:, 0, 0:CHR]

            def stage_chunk(src, row0, eng=None):
                """bf16-transposed [128, NCH, CHR] chunk of src rows.

                Loads go on SP (or gpsimd for the late wave); transposes go on
                the Activation queue so SP's in-order queue never serializes
                load(n+1) behind transpose(n)'s wait on the DVE cast.
                """
                eng = eng or nc.sync
                xf = xs.tile([128, CH_RT, HID], F32, tag="xf")
                eng.dma_start(
                    xf[:],
                    src[row0:row0 + CHR, :].rearrange("(t p) c -> p t c",
                                                      p=128))
                xc = xb.tile([128, CH_RT, HID], BF16, tag="xc")
                nc.vector.tensor_copy(xc[:], xf[:])
                # single xbar transpose of the flattened [128, 2048] block:
                # out[:, t, hc, r] = rows row0+t*128+r of hidden chunk hc
                xT = xbt.tile([128, CH_RT, NCH, 128], BF16, tag="xT")
                nc.scalar.dma_start_transpose(
                    xT[:].rearrange("p t h r -> p (t h) r"),
                    xc[:].rearrange("p t c -> p (t c)"))
                return xT

            def load_w_bf16(wdram, stage=None):
                """Load+cast w in quarters, optionally interleaving the
                staging of x chunks between quarters so the DMA device
                alternates weight and activation traffic at startup."""
                quarters = []
                for qq in range(4):
                    wf = ws.tile([128, 2, HID], F32, tag="wf")
                    nc.gpsimd.dma_start(
                        wf[:],
                        wdram[qq * CHR:(qq + 1) * CHR, :].rearrange(
                            "(t p) c -> p t c", p=128))
                    wh = wb.tile([128, 2, HID], BF16, tag="wb")
                    nc.scalar.copy(wh[:], wf[:])
                    quarters.append(wh)
                    if stage is not None:
                        stage(qq)
                return quarters

            def wsl(quarters, hc, csl):
                return quarters[hc // 2][:, hc % 2, csl]

            # K projection -> kTh (transposed strips), no redundancy.
            k_xT = []
            wk_h = load_w_bf16(
                w_k, stage=lambda c: k_xT.append(stage_chunk(x_k, c * CHR)))
            for c in range(KLOC // CHR):
                for s in range(NCH):
                    ps = proj_ps(f"psk_{c}_{s}")
                    for hc in range(NCH):
                        nc.tensor.matmul(
                            ps,
                            wsl(wk_h, hc, slice(s * 128, (s + 1) * 128)),
                            k_xT[c][:, :, hc, :],
                            start=(hc == 0), stop=(hc == NCH - 1))
                    nc.vector.tensor_copy(
                        kTh[:, s, c * CHR:(c + 1) * CHR], ps)

            # V projection -> v_aug (natural layout), no redundancy.
            v_xT = []
            wv_h = load_w_bf16(
                w_v, stage=lambda c: v_xT.append(stage_chunk(x_v, c * CHR)))
            for c in range(KLOC // CHR):
                for t in range(CH_RT):
                    kb = c * CH_RT + t
                    for sg in range(4):  # 256-col groups = 4 heads each
                        ps = proj_ps(f"psv_{kb}_{sg}")
                        for hc in range(NCH):
                            nc.tensor.matmul(
                                ps,
                                v_xT[c][:, t, hc, :],
                                wsl(wv_h, hc, slice(sg * 256, (sg + 1) * 256)),
                                start=(hc == 0), stop=(hc == NCH - 1))
                        nc.vector.tensor_copy(
                            v_aug[:, kb, 4 * sg:4 * sg + 4, 0:HD],
                            ps.rearrange("p (h d) -> p h d", h=4))

            # Q staging wave 1 (chunks 0-3, SP queue); wave 2 (4-7) issues on
            # the gpsimd queue mid-attention so its DMAs don't sit behind the
            # attention ctx stores on SP.
            q_xT = []
            wq_h = load_w_bf16(
                w_q, stage=lambda c: q_xT.append(stage_chunk(x_q, c * CHR)))

            def qproj_chunk(c):
                for s in range(NCH):
                    ps = proj_ps(f"psq_{c}_{s}")
                    for hc in range(NCH):
                        nc.tensor.matmul(
                            ps,
                            wsl(wq_h, hc, slice(s * 128, (s + 1) * 128)),
                            q_xT[c][:, :, hc, :],
                            start=(hc == 0), stop=(hc == NCH - 1))
                    nc.vector.tensor_copy(
                        qTh[:, s, c * CHR:(c + 1) * CHR], ps)

            # ---------------- attention + RS + output ----------------
            att_st = ExitStack()
            ctxp = att_st.enter_context(
                tc.tile_pool(name="pctx", bufs=2, space="PSUM"))
            apool = att_st.enter_context(tc.tile_pool(name="apool", bufs=4))
            csb = att_st.enter_context(tc.tile_pool(name="ctxsb", bufs=2))
            wop = att_st.enter_context(tc.tile_pool(name="wo", bufs=1))
            fp = att_st.enter_context(tc.tile_pool(name="fragp", bufs=1))
            rp = att_st.enter_context(tc.tile_pool(name="normp", bufs=1))
            cnp = att_st.enter_context(tc.tile_pool(name="ctxNp", bufs=4))

            def qproj_strip(c, s):
                ps = proj_ps(f"pss_{c}_{s}")
                for hc in range(NCH):
                    nc.tensor.matmul(
                        ps,
                        wsl(wq_h, hc, slice(s * 128, (s + 1) * 128)),
                        q_xT[c][:, :, hc, :],
                        start=(hc == 0), stop=(hc == NCH - 1))
                nc.vector.tensor_copy(
                    qTh[:, s, c * CHR:(c + 1) * CHR], ps)

            def attn_qb(qb, next_chunks=None):
                qsl = slice(qb * QB, (qb + 1) * QB)
                for h in range(HEADS):
                    # Interleave one q-proj strip for the next q-block per
                    # head: keeps Act's exp stream dense across qb boundaries
                    # by filling PE's slack instead of a serial proj block.
                    if next_chunks is not None:
                        nc_c = next_chunks[h // NCH]
                        qproj_strip(nc_c, h % NCH)
                    s, i = h // 2, h % 2
                    psl = slice(i * HD, (i + 1) * HD)
                    ctx = ctxp.tile([HD + 1, QB], F32, tag="ctx",
                                    name=f"ctx_{qb}_{h}")
                    sts = []
                    As = []

                    def scores(g):
                        st = stp.tile([128, GKB, QB], F32, tag="st",
                                      name=f"st_{qb}_{h}_{g}")
                        for j in range(GKB):
                            kb = g * GKB + j
                            nc.tensor.matmul(
                                st[:, j, :],
                                kTh[psl, s, kb * 128:(kb + 1) * 128],
                                qTh[psl, s, qsl],
                                start=True, stop=True)
                        sts.append(st)
                        a = apool.tile([128, GKB, QB], BF16, tag="a",
                                       name=f"a_{qb}_{h}_{g}")
                        nc.scalar.activation(
                            a[:], st[:], mybir.ActivationFunctionType.Exp,
                            scale=SCALE)
                        As.append(a)

                    def pv(g):
                        for j in range(GKB):
                            kb = g * GKB + j
                            nc.tensor.matmul(
                                ctx[:],
                                v_aug[:, kb, h, :],
                                As[g][:, j, :],
                                start=(kb == 0), stop=(kb == NKB - 1))

                    ngr = NKB // GKB
                    scores(0)
                    scores(1)
                    scores(2)
                    for g in range(ngr):
                        if g + 3 < ngr:
                            scores(g + 3)
                        pv(g)
                    ctx_sb = csb.tile([HD + 1, QB], BF16, tag="ctx_sb",
                                      name=f"ctxsb_{qb}_{h}")
                    nc.vector.tensor_copy(ctx_sb[:], ctx[:])
                    nc.sync.dma_start(
                        ctx_part[qb][h // seg_h][:, h % seg_h, :, :].rearrange(
                            "a p b -> p a b"),
                        ctx_sb[:].rearrange("p (a b) -> p a b", a=4))
                    if (h + 1) % seg_h == 0:
                        rs_qb(qb, h // seg_h)

            RGROUPS = [[0, 1, 2, 3], [4, 5, 6, 7]]

            def rs_qb(qb, seg):
                if rs_mode == "none":
                    return  # timing-only floor: skip collectives
                if rs_mode == "a2a":
                    nc.gpsimd.collective_compute(
                        "AllToAll", mybir.AluOpType.bypass,
                        ins=[ctx_part[qb][seg][:]],
                        outs=[a2a_out[qb][:]],
                        replica_groups=RGROUPS)
                    return
                nc.gpsimd.collective_compute(
                    "ReduceScatter", mybir.AluOpType.add,
                    ins=[ctx_part[qb][seg][:]],
                    outs=[frag[qb][seg * seg_h:(seg + 1) * seg_h]],
                    replica_groups=RGROUPS)

            state = {}

            def load_wo():
                wo_sb = wop.tile([128, NCH, HID], BF16)
                for qq in range(4):
                    wf = ws.tile([128, 2, HID], F32, tag="wf")
                    for j in range(2):
                        hc = qq * 2 + j
                        nc.gpsimd.dma_start(wf[:, j, :],
                                            w_o[hc * 128:(hc + 1) * 128, :])
                    nc.vector.tensor_copy(
                        wo_sb[:, qq * 2:qq * 2 + 2, :], wf[:])
                state["wo_sb"] = wo_sb

            ctxNs = {}

            def norm_qb(qb):
                fsb = fp.tile([HD + 1, HEADS, 128], BF16, tag="fsb")
                if rs_mode == "a2a":
                    # sum the 4 source blocks routed to us by the AllToAll
                    fsc = fp.tile([HD + 1, HEADS, 128], BF16, tag="fsc")
                    nc.gpsimd.dma_start(
                        fsb[:], a2a_out[qb][0].rearrange("h p b -> p h b"))
                    for src in range(1, 4):
                        nc.gpsimd.dma_start(
                            fsc[:],
                            a2a_out[qb][src].rearrange("h p b -> p h b"))
                        nc.vector.tensor_add(fsb[:], fsb[:], fsc[:])
                else:
                    nc.gpsimd.dma_start(
                        fsb[:], frag[qb][:].rearrange("h p b -> p h b"))
                ctxN = cnp.tile([128, NCH, 128], BF16, tag="ctxN",
                                name=f"ctxN_{qb}")
                for hf in range(2):  # 8-head halves
                    hsl = slice(hf * 8, (hf + 1) * 8)
                    rinv = rp.tile([1, 8, 128], F32, tag="rinv",
                                   name=f"rinv_{qb}_{hf}")
                    nc.vector.reciprocal(rinv[:], fsb[HD:HD + 1, hsl, :])
                    rb = rp.tile([HD, 8, 128], F32, tag="rb",
                                 name=f"rb_{qb}_{hf}")
                    nc.gpsimd.partition_broadcast(rb[:], rinv[:])
                    fsb_r = fsb[0:HD, hsl, :].rearrange(
                        "p (c i) b -> p c i b", i=2)
                    rb_r = rb[:].rearrange("p (c i) b -> p c i b", i=2)
                    for i in range(2):
                        nc.vector.tensor_mul(
                            ctxN[i * HD:(i + 1) * HD, hf * 4:(hf + 1) * 4, :],
                            fsb_r[:, :, i, :],
                            rb_r[:, :, i, :])
                ctxNs[qb] = ctxN

            def out_qb(qb):
                wo_sb = state["wo_sb"]
                ctxN = ctxNs[qb]
                po_sb = fp.tile([128, HID], F32, tag="po_sb")
                po = stp.tile([128, GKB, QB], F32, tag="st",
                              name=f"po_{qb}")
                for half in range(2):
                    osl = slice(half * QB, (half + 1) * QB)
                    for hc in range(NCH):
                        nc.tensor.matmul(po[:, half, :], ctxN[:, hc, :],
                                         wo_sb[:, hc, osl],
                                         start=(hc == 0), stop=(hc == NCH - 1))
                nc.vector.tensor_copy(
                    po_sb[:], po[:].rearrange("p a b -> p (a b)"))
                nc.sync.dma_start(out_frag[qb], po_sb[:])

            # schedule: q-proj strips for block qb+1 are interleaved inside
            # attention block qb (one strip per head); RS issues per head-half.
            qproj_chunk(0)
            qproj_chunk(1)
            attn_qb(0, next_chunks=(2, 3))
            for c in range(4, 8):
                q_xT.append(stage_chunk(x_q, c * CHR, eng=nc.gpsimd))
            load_wo()
            attn_qb(1, next_chunks=(4, 5))
            norm_qb(0)
            attn_qb(2, next_chunks=(6, 7))
            norm_qb(1)
            attn_qb(3)
            norm_qb(2)
            out_qb(0)
            out_qb(1)
            out_qb(2)
            norm_qb(3)
            out_qb(3)
            att_st.close()
            proj_st.close()
            est.close()

    nc.compile()
    return nc


def _get_nc():
    global _CACHED_NC
    if _CACHED_NC is None:
        _CACHED_NC = _build()
    return _CACHED_NC


def make_in_maps(query, key, value, w_q, w_k, w_v, w_o):
    ins = []
    for core in range(NCORE):
        b, kq = core // 4, core % 4
        ins.append({
            "x_q": np.ascontiguousarray(query[b]),
            "x_k": np.ascontiguousarray(key[b][kq * KLOC:(kq + 1) * KLOC]),
            "x_v": np.ascontiguousarray(value[b][kq * KLOC:(kq + 1) * KLOC]),
            "w_q": np.ascontiguousarray(w_q),
            "w_k": np.ascontiguousarray(w_k),
            "w_v": np.ascontiguousarray(w_v),
            "w_o": np.ascontiguousarray(w_o),
        })
    return ins


def assemble(results):
    out = np.empty((B, Q, HID), np.float32)
    for core in range(NCORE):
        b, kq = core // 4, core % 4
        fragr = results[core]["out_frag"]
        for qb in range(NQB):
            r0 = qb * QB + kq * 128
            out[b, r0:r0 + 128, :] = fragr[qb]
    return out


_EXEC = None


def _get_exec():
    """Build the 8-core shard_map executable once; reuse across calls."""
    global _EXEC
    if _EXEC is not None:
        return _EXEC
    import jax
    from jax.sharding import Mesh, PartitionSpec
    from jax.experimental.shard_map import shard_map
    from concourse.bass2jax import (_bass_exec_p, install_neuronx_cc_hook,
                                    partition_id_tensor)

    install_neuronx_cc_hook()
    nc = _get_nc()
    in_names, out_names, out_avals, zero_outs = [], [], [], []
    for alloc in nc.m.functions[0].allocations:
        if not isinstance(alloc, mybir.MemoryLocationSet):
            continue
        name = alloc.memorylocations[0].name
        if alloc.kind == "ExternalInput":
            if name != "partition_id":
                in_names.append(name)
        elif alloc.kind == "ExternalOutput":
            out_names.append(name)
            shape = tuple(alloc.tensor_shape)
            dtype = mybir.dt.np(alloc.dtype)
            out_avals.append(jax.core.ShapedArray(shape, dtype))
            zero_outs.append(np.zeros(shape, dtype))
    partition_name = (nc.partition_id_tensor.name
                      if nc.partition_id_tensor else None)
    all_in = list(in_names) + list(out_names)
    if partition_name:
        all_in.append(partition_name)

    def _body(*args):
        operands = list(args)
        if partition_name is not None:
            operands.append(partition_id_tensor())
        return tuple(_bass_exec_p.bind(
            *operands, out_avals=tuple(out_avals), in_names=tuple(all_in),
            out_names=tuple(out_names), lowering_input_output_aliases=(),
            sim_require_finite=True, sim_require_nnan=True, nc=nc))

    devices = jax.devices()[:NCORE]
    mesh = Mesh(np.asarray(devices), ("core",))
    n_all = len(in_names) + len(out_names)
    fn = jax.jit(shard_map(_body, mesh=mesh,
                           in_specs=(PartitionSpec("core"),) * n_all,
                           out_specs=(PartitionSpec("core"),) * len(out_names),
                           check_rep=False), keep_unused=True)
    concat_zeros = [np.zeros((NCORE * z.shape[0], *z.shape[1:]), z.dtype)
                    for z in zero_outs]
    _EXEC = (fn, in_names, out_names, out_avals, concat_zeros)
    return _EXEC


def kernel(query, key, value, w_q, w_k, w_v, w_o):
    query = np.asarray(query, dtype=np.float32)
    key = np.asarray(key, dtype=np.float32)
    value = np.asarray(value, dtype=np.float32)
    ins = make_in_maps(query, key, value, np.asarray(w_q, np.float32),
                       np.asarray(w_k, np.float32), np.asarray(w_v, np.float32),
                       np.asarray(w_o, np.float32))
    fn, in_names, out_names, out_avals, concat_zeros = _get_exec()
    concat_in = [np.concatenate([np.asarray(ins[c][nm]) for c in range(NCORE)])
                 for nm in in_names]
    out_arrs = fn(*concat_in, *concat_zeros)
    results = [
        {nm: np.asarray(out_arrs[i]).reshape(NCORE, *out_avals[i].shape)[c]
         for i, nm in enumerate(out_names)}
        for c in range(NCORE)]
    return assemble(results)


if __name__ == "__main__":
    np.random.seed(0)
    q = np.random.randn(B, Q, HID).astype(np.float32)
    k = np.random.randn(B, KL, HID).astype(np.float32)
    v = np.random.randn(B, KL, HID).astype(np.float32)
    s = 1.0 / np.sqrt(HID)
    wq = (np.random.randn(HID, HID) * s).astype(np.float32)
    wk = (np.random.randn(HID, HID) * s).astype(np.float32)
    wv = (np.random.randn(HID, HID) * s).astype(np.float32)
    wo = (np.random.randn(HID, HID) * s).astype(np.float32)
    t0 = time.time()
    out = kernel(q, k, v, wq, wk, wv, wo)
    print("kernel done", time.time() - t0, out.shape)


# revision 58
# speedup vs baseline: 1.4022x; 1.4022x over previous
"""Trainium2 Bass kernel for MemoryEfficientCrossAttention (seq-parallel v2).

Problem (hardcoded): B=2, Q=2048, K=4096, HIDDEN=1024, HEADS=16, HEAD_DIM=64.
  out = softmax((x_q W_q)(x_k W_k)^T / sqrt(64)) (x_v W_v) W_o

Sharding over 8 NeuronCores: core = b*4 + kq
  b in {0,1}: batch;  kq in {0..3}: K-quarter (1024 contiguous k rows)
Each core projects q for ALL 2048 rows of its batch (4x redundant but cheap),
k/v for its local 1024 k rows (no redundancy), computes unnormalized
attention ctx^T (with a ones-column for the softmax denominators) against its
local k, then a 4-core ReduceScatter per 512-q block sums the partial ctx and
scatters 128-q fragments; each core normalizes its fragments and applies W_o
for its 4x128 output rows.  No other collectives.

Precision: activations and weights are cast to bf16 for the QKV projections
and attention matmuls (psum accumulation stays f32); the xbar DMA-transpose of
x runs on the single bf16 plane.  W_o path stays f32r.  Expected rel err
~5e-3 versus the f32 reference (gate is 2e-2).
"""

import os
import sys
import time

import numpy as np

sys.path.insert(0, "/opt/trn_rl_repo")

import concourse.bass as bass  # noqa: E402
import concourse.mybir as mybir  # noqa: E402
import concourse.tile as tile  # noqa: E402
from concourse import bacc  # noqa: E402

F32 = mybir.dt.float32
F32R = mybir.dt.float32r
BF16 = mybir.dt.bfloat16

HID = 1024
HEADS = 16
HD = 64
B = 2
Q = 2048
KL = 4096
NCORE = 8
KLOC = KL // 4        # local k rows per core
NCH = HID // 128      # 8 hidden chunks
CH_RT = 2             # 128-row tiles per transpose chunk
CHR = CH_RT * 128     # 256 rows per chunk
NKB = KLOC // 128     # 8 local k-blocks
NQB = 4               # 512-q blocks
QB = Q // NQB         # 512
GKB = 2               # k-blocks per score/exp group
SCALE = HD ** -0.5

_CACHED_NC = None


def _build():
    nc = bacc.Bacc("TRN2", target_bir_lowering=False, debug=False,
                   num_devices=NCORE)

    x_q = nc.dram_tensor("x_q", [Q, HID], F32, kind="ExternalInput")
    x_k = nc.dram_tensor("x_k", [KLOC, HID], F32, kind="ExternalInput")
    x_v = nc.dram_tensor("x_v", [KLOC, HID], F32, kind="ExternalInput")
    w_q = nc.dram_tensor("w_q", [HID, HID], F32, kind="ExternalInput")
    w_k = nc.dram_tensor("w_k", [HID, HID], F32, kind="ExternalInput")
    w_v = nc.dram_tensor("w_v", [HID, HID], F32, kind="ExternalInput")
    w_o = nc.dram_tensor("w_o", [HID, HID], F32, kind="ExternalInput")
    out_frag = nc.dram_tensor("out_frag", [NQB, 128, HID], F32,
                              kind="ExternalOutput")

    from contextlib import ExitStack

    with tile.TileContext(nc, pool_alloc_mode="queue") as tc:
        with tc.tile_pool(name="dram", bufs=1, space="DRAM") as dram:
            est = ExitStack()
            pp = est.enter_context(tc.tile_pool(name="persist", bufs=1))
            # partial ctx^T per 512-q block:
            # [head-half, qquad, head%8, d(+denom), 128 q]
            shared = "Shared" if os.environ.get("KSHARED", "0") == "1" \
                else "Local"
            rs_mode = os.environ.get("KRS", "half")
            # seg_h heads per collective segment; each segment is its own
            # contiguous DRAM tensor (BIR requires contiguous collective in).
            seg_h = HEADS if rs_mode in ("full", "a2a") else                 int(os.environ.get("KSEG", "8"))
            nseg = HEADS // seg_h
            ctx_part = [[dram.tile([4, seg_h, HD + 1, 128], BF16,
                                   name=f"ctx_part{i}_{s}")
                         for s in range(nseg)]
                        for i in range(NQB)]
            a2a_out = [dram.tile([4, HEADS, HD + 1, 128], BF16,
                                 name=f"a2a_out{i}")
                       for i in range(NQB)]
            frag = [dram.tile([HEADS, HD + 1, 128], BF16, name=f"frag{i}",
                              addr_space=shared)
                    for i in range(NQB)]

            qTh = pp.tile([128, NCH, Q], BF16)      # strip s = heads 2s,2s+1
            kTh = pp.tile([128, NCH, KLOC], BF16)
            v_aug = pp.tile([128, NKB, HEADS, HD + 1], BF16)

            ones = pp.tile([128, NKB * HEADS], BF16, name="ones")
            nc.vector.memset(ones[:], 1.0)
            nc.vector.tensor_copy(
                v_aug[:, :, :, HD],
                ones[:].rearrange("p (a b) -> p a b", a=NKB))

            # ---------------- staging + projections ----------------
            proj_st = ExitStack()
            xs = proj_st.enter_context(tc.tile_pool(name="xstage", bufs=2))
            xb = proj_st.enter_context(tc.tile_pool(name="xbf", bufs=3))
            xbt = proj_st.enter_context(tc.tile_pool(name="xT", bufs=4))
            ws = proj_st.enter_context(tc.tile_pool(name="wstage", bufs=1))
            wb = proj_st.enter_context(tc.tile_pool(name="wbf", bufs=8))
            pj = proj_st.enter_context(
                tc.tile_pool(name="pproj", bufs=2, space="PSUM"))

            def proj_ps(name):
                t = pj.tile([128, CHR], F32, tag="ps", name=name)
                return t[:]

            def stage_chunk(src, row0, eng=None):
                """bf16-transposed [128, NCH, CHR] chunk of src rows.

                Loads go on SP (or gpsimd for the late wave); transposes go on
                the Activation queue so SP's in-order queue never serializes
                load(n+1) behind transpose(n)'s wait on the DVE cast.
                """
                eng = eng or nc.sync
                xf = xs.tile([128, CH_RT, HID], F32, tag="xf")
                eng.dma_start(
                    xf[:],
                    src[row0:row0 + CHR, :].rearrange("(t p) c -> p t c",
                                                      p=128))
                xc = xb.tile([128, CH_RT, HID], BF16, tag="xc")
                nc.vector.tensor_copy(xc[:], xf[:])
                # single xbar transpose of the flattened [128, 2048] block:
                # out[:, t, hc, r] = rows row0+t*128+r of hidden chunk hc
                xT = xbt.tile([128, CH_RT, NCH, 128], BF16, tag="xT")
                nc.scalar.dma_start_transpose(
                    xT[:].rearrange("p t h r -> p (t h) r"),
                    xc[:].rearrange("p t c -> p (t c)"))
                return xT

            def load_w_bf16(wdram, stage=None):
                """Load+cast w in quarters, optionally interleaving the
                staging of x chunks between quarters so the DMA device
                alternates weight and activation traffic at startup."""
                quarters = []
                for qq in range(4):
                    wf = ws.tile([128, 2, HID], F32, tag="wf")
                    nc.gpsimd.dma_start(
                        wf[:],
                        wdram[qq * CHR:(qq + 1) * CHR, :].rearrange(
                            "(t p) c -> p t c", p=128))
                    wh = wb.tile([128, 2, HID], BF16, tag="wb")
                    nc.scalar.copy(wh[:], wf[:])
                    quarters.append(wh)
                    if stage is not None:
                        stage(qq)
                return quarters

            def wsl(quarters, hc, csl):
                return quarters[hc // 2][:, hc % 2, csl]

            # K projection -> kTh (transposed strips), no redundancy.
            k_xT = []
            wk_h = load_w_bf16(
                w_k, stage=lambda c: k_xT.append(stage_chunk(x_k, c * CHR)))
            for c in range(KLOC // CHR):
                for s in range(NCH):
                    ps = proj_ps(f"psk_{c}_{s}")
                    for hc in range(NCH):
                        nc.tensor.matmul(
                            ps,
                            wsl(wk_h, hc, slice(s * 128, (s + 1) * 128)),
                            k_xT[c][:, :, hc, :],
                            start=(hc == 0), stop=(hc == NCH - 1))
                    nc.vector.tensor_copy(
                        kTh[:, s, c * CHR:(c + 1) * CHR], ps)

            # V projection -> v_aug (natural layout), no redundancy.
            v_xT = []
            wv_h = load_w_bf16(
                w_v, stage=lambda c: v_xT.append(stage_chunk(x_v, c * CHR)))
            for c in range(KLOC // CHR):
                for t in range(CH_RT):
                    kb = c * CH_RT + t
                    for sg in range(4):  # 256-col groups = 4 heads each
                        ps = proj_ps(f"psv_{kb}_{sg}")
                        for hc in range(NCH):
                            nc.tensor.matmul(
                                ps,
                                v_xT[c][:, t, hc, :],
                                wsl(wv_h, hc, slice(sg * 256, (sg + 1) * 256)),
                                start=(hc == 0), stop=(hc == NCH - 1))
                        nc.vector.tensor_copy(
                            v_aug[:, kb, 4 * sg:4 * sg + 4, 0:HD],
                            ps.rearrange("p (h d) -> p h d", h=4))

            # Q staging wave 1 (chunks 0-3, SP queue); wave 2 (4-7) issues on
            # the gpsimd queue mid-attention so its DMAs don't sit behind the
            # attention ctx stores on SP.
            q_xT = []
            wq_h = load_w_bf16(
                w_q, stage=lambda c: q_xT.append(stage_chunk(x_q, c * CHR)))

            def qproj_chunk(c):
                for s in range(NCH):
                    ps = proj_ps(f"psq_{c}_{s}")
                    for hc in range(NCH):
                        nc.tensor.matmul(
                            ps,
                            wsl(wq_h, hc, slice(s * 128, (s + 1) * 128)),
                            q_xT[c][:, :, hc, :],
                            start=(hc == 0), stop=(hc == NCH - 1))
                    nc.vector.tensor_copy(
                        qTh[:, s, c * CHR:(c + 1) * CHR], ps)

            # ---------------- attention + RS + output ----------------
            att_st = ExitStack()
            stp = att_st.enter_context(
                tc.tile_pool(name="pscore", bufs=2, space="PSUM"))
            ctxp = att_st.enter_context(
                tc.tile_pool(name="pctx", bufs=2, space="PSUM"))
            apool = att_st.enter_context(tc.tile_pool(name="apool", bufs=3))
            csb = att_st.enter_context(tc.tile_pool(name="ctxsb", bufs=2))
            wop = att_st.enter_context(tc.tile_pool(name="wo", bufs=1))
            fp = att_st.enter_context(tc.tile_pool(name="fragp", bufs=1))
            rp = att_st.enter_context(tc.tile_pool(name="normp", bufs=1))
            cnp = att_st.enter_context(tc.tile_pool(name="ctxNp", bufs=4))

            def qproj_strip(c, s):
                ps = proj_ps(f"pss_{c}_{s}")
                for hc in range(NCH):
                    nc.tensor.matmul(
                        ps,
                        wsl(wq_h, hc, slice(s * 128, (s + 1) * 128)),
                        q_xT[c][:, :, hc, :],
                        start=(hc == 0), stop=(hc == NCH - 1))
                nc.vector.tensor_copy(
                    qTh[:, s, c * CHR:(c + 1) * CHR], ps)

            def attn_qb(qb, next_chunks=None):
                qsl = slice(qb * QB, (qb + 1) * QB)
                for h in range(HEADS):
                    # Interleave one q-proj strip for the next q-block per
                    # head: keeps Act's exp stream dense across qb boundaries
                    # by filling PE's slack instead of a serial proj block.
                    if next_chunks is not None:
                        nc_c = next_chunks[h // NCH]
                        qproj_strip(nc_c, h % NCH)
                    s, i = h // 2, h % 2
                    psl = slice(i * HD, (i + 1) * HD)
                    ctx = ctxp.tile([HD + 1, QB], F32, tag="ctx",
                                    name=f"ctx_{qb}_{h}")
                    sts = []
                    As = []

                    def scores(g):
                        st = stp.tile([128, GKB, QB], F32, tag="st",
                                      name=f"st_{qb}_{h}_{g}")
                        for j in range(GKB):
                            kb = g * GKB + j
                            nc.tensor.matmul(
                                st[:, j, :],
                                kTh[psl, s, kb * 128:(kb + 1) * 128],
                                qTh[psl, s, qsl],
                                start=True, stop=True)
                        sts.append(st)
                        a = apool.tile([128, GKB, QB], BF16, tag="a",
                                       name=f"a_{qb}_{h}_{g}")
                        nc.scalar.activation(
                            a[:], st[:], mybir.ActivationFunctionType.Exp,
                            scale=SCALE)
                        As.append(a)

                    def pv(g):
                        for j in range(GKB):
                            kb = g * GKB + j
                            nc.tensor.matmul(
                                ctx[:],
                                v_aug[:, kb, h, :],
                                As[g][:, j, :],
                                start=(kb == 0), stop=(kb == NKB - 1))

                    ngr = NKB // GKB
                    scores(0)
                    scores(1)
                    for g in range(ngr):
                        if g + 2 < ngr:
                            scores(g + 2)
                        pv(g)
                    ctx_sb = csb.tile([HD + 1, QB], BF16, tag="ctx_sb",
                                      name=f"ctxsb_{qb}_{h}")
                    nc.vector.tensor_copy(ctx_sb[:], ctx[:])
                    nc.sync.dma_start(
                        ctx_part[qb][h // seg_h][:, h % seg_h, :, :].rearrange(
                            "a p b -> p a b"),
                        ctx_sb[:].rearrange("p (a b) -> p a b", a=4))
                    if (h + 1) % seg_h == 0:
                        rs_qb(qb, h // seg_h)

            RGROUPS = [[0, 1, 2, 3], [4, 5, 6, 7]]

            def rs_qb(qb, seg):
                if rs_mode == "none":
                    return  # timing-only floor: skip collectives
                if rs_mode == "a2a":
                    nc.gpsimd.collective_compute(
                        "AllToAll", mybir.AluOpType.bypass,
                        ins=[ctx_part[qb][seg][:]],
                        outs=[a2a_out[qb][:]],
                        replica_groups=RGROUPS)
                    return
                nc.gpsimd.collective_compute(
                    "ReduceScatter", mybir.AluOpType.add,
                    ins=[ctx_part[qb][seg][:]],
                    outs=[frag[qb][seg * seg_h:(seg + 1) * seg_h]],
                    replica_groups=RGROUPS)

            state = {}

            def load_wo():
                wo_sb = wop.tile([128, NCH, HID], BF16)
                for qq in range(4):
                    wf = ws.tile([128, 2, HID], F32, tag="wf")
                    for j in range(2):
                        hc = qq * 2 + j
                        nc.gpsimd.dma_start(wf[:, j, :],
                                            w_o[hc * 128:(hc + 1) * 128, :])
                    nc.vector.tensor_copy(
                        wo_sb[:, qq * 2:qq * 2 + 2, :], wf[:])
                state["wo_sb"] = wo_sb

            ctxNs = {}

            def norm_qb(qb):
                fsb = fp.tile([HD + 1, HEADS, 128], BF16, tag="fsb")
                if rs_mode == "a2a":
                    # sum the 4 source blocks routed to us by the AllToAll
                    fsc = fp.tile([HD + 1, HEADS, 128], BF16, tag="fsc")
                    nc.gpsimd.dma_start(
                        fsb[:], a2a_out[qb][0].rearrange("h p b -> p h b"))
                    for src in range(1, 4):
                        nc.gpsimd.dma_start(
                            fsc[:],
                            a2a_out[qb][src].rearrange("h p b -> p h b"))
                        nc.vector.tensor_add(fsb[:], fsb[:], fsc[:])
                else:
                    nc.gpsimd.dma_start(
                        fsb[:], frag[qb][:].rearrange("h p b -> p h b"))
                ctxN = cnp.tile([128, NCH, 128], BF16, tag="ctxN",
                                name=f"ctxN_{qb}")
                for hf in range(2):  # 8-head halves
                    hsl = slice(hf * 8, (hf + 1) * 8)
                    rinv = rp.tile([1, 8, 128], F32, tag="rinv",
                                   name=f"rinv_{qb}_{hf}")
                    nc.vector.reciprocal(rinv[:], fsb[HD:HD + 1, hsl, :])
                    rb = rp.tile([HD, 8, 128], F32, tag="rb",
                                 name=f"rb_{qb}_{hf}")
                    nc.gpsimd.partition_broadcast(rb[:], rinv[:])
                    fsb_r = fsb[0:HD, hsl, :].rearrange(
                        "p (c i) b -> p c i b", i=2)
                    rb_r = rb[:].rearrange("p (c i) b -> p c i b", i=2)
                    for i in range(2):
                        nc.vector.tensor_mul(
                            ctxN[i * HD:(i + 1) * HD, hf * 4:(hf + 1) * 4, :],
                            fsb_r[:, :, i, :],
                            rb_r[:, :, i, :])
                ctxNs[qb] = ctxN

            def out_qb(qb):
                wo_sb = state["wo_sb"]
                ctxN = ctxNs[qb]
                po_sb = fp.tile([128, HID], F32, tag="po_sb")
                po = stp.tile([128, GKB, QB], F32, tag="st",
                              name=f"po_{qb}")
                for half in range(2):
                    osl = slice(half * QB, (half + 1) * QB)
                    for hc in range(NCH):
                        nc.tensor.matmul(po[:, half, :], ctxN[:, hc, :],
                                         wo_sb[:, hc, osl],
                                         start=(hc == 0), stop=(hc == NCH - 1))
                nc.vector.tensor_copy(
                    po_sb[:], po[:].rearrange("p a b -> p (a b)"))
                nc.sync.dma_start(out_frag[qb], po_sb[:])

            # schedule: q-proj strips for block qb+1 are interleaved inside
            # attention block qb (one strip per head); RS issues per head-half.
            qproj_chunk(0)
            qproj_chunk(1)
            attn_qb(0, next_chunks=(2, 3))
            for c in range(4, 8):
                q_xT.append(stage_chunk(x_q, c * CHR, eng=nc.gpsimd))
            load_wo()
            attn_qb(1, next_chunks=(4, 5))
            norm_qb(0)
            attn_qb(2, next_chunks=(6, 7))
            norm_qb(1)
            attn_qb(3)
            norm_qb(2)
            out_qb(0)
            out_qb(1)
            out_qb(2)
            norm_qb(3)
            out_qb(3)
            att_st.close()
            proj_st.close()
            est.close()

    nc.compile()
    return nc


def _get_nc():
    global _CACHED_NC
    if _CACHED_NC is None:
        _CACHED_NC = _build()
    return _CACHED_NC


def make_in_maps(query, key, value, w_q, w_k, w_v, w_o):
    ins = []
    for core in range(NCORE):
        b, kq = core // 4, core % 4
        ins.append({
            "x_q": np.ascontiguousarray(query[b]),
            "x_k": np.ascontiguousarray(key[b][kq * KLOC:(kq + 1) * KLOC]),
            "x_v": np.ascontiguousarray(value[b][kq * KLOC:(kq + 1) * KLOC]),
            "w_q": np.ascontiguousarray(w_q),
            "w_k": np.ascontiguousarray(w_k),
            "w_v": np.ascontiguousarray(w_v),
            "w_o": np.ascontiguousarray(w_o),
        })
    return ins


def assemble(results):
    out = np.empty((B, Q, HID), np.float32)
    for core in range(NCORE):
        b, kq = core // 4, core % 4
        fragr = results[core]["out_frag"]
        for qb in range(NQB):
            r0 = qb * QB + kq * 128
            out[b, r0:r0 + 128, :] = fragr[qb]
    return out


_EXEC = None


def _get_exec():
    """Build the 8-core shard_map executable once; reuse across calls."""
    global _EXEC
    if _EXEC is not None:
        return _EXEC
    import jax
    from jax.sharding import Mesh, PartitionSpec
    from jax.experimental.shard_map import shard_map
    from concourse.bass2jax import (_bass_exec_p, install_neuronx_cc_hook,
                                    partition_id_tensor)

    install_neuronx_cc_hook()
    nc = _get_nc()
    in_names, out_names, out_avals, zero_outs = [], [], [], []
    for alloc in nc.m.functions[0].allocations:
        if not isinstance(alloc, mybir.MemoryLocationSet):
            continue
        name = alloc.memorylocations[0].name
        if alloc.kind == "ExternalInput":
            if name != "partition_id":
                in_names.append(name)
        elif alloc.kind == "ExternalOutput":
            out_names.append(name)
            shape = tuple(alloc.tensor_shape)
            dtype = mybir.dt.np(alloc.dtype)
            out_avals.append(jax.core.ShapedArray(shape, dtype))
            zero_outs.append(np.zeros(shape, dtype))
    partition_name = (nc.partition_id_tensor.name
                      if nc.partition_id_tensor else None)
    all_in = list(in_names) + list(out_names)
    if partition_name:
        all_in.append(partition_name)

    def _body(*args):
        operands = list(args)
        if partition_name is not None:
            operands.append(partition_id_tensor())
        return tuple(_bass_exec_p.bind(
            *operands, out_avals=tuple(out_avals), in_names=tuple(all_in),
            out_names=tuple(out_names), lowering_input_output_aliases=(),
            sim_require_finite=True, sim_require_nnan=True, nc=nc))

    devices = jax.devices()[:NCORE]
    mesh = Mesh(np.asarray(devices), ("core",))
    n_all = len(in_names) + len(out_names)
    fn = jax.jit(shard_map(_body, mesh=mesh,
                           in_specs=(PartitionSpec("core"),) * n_all,
                           out_specs=(PartitionSpec("core"),) * len(out_names),
                           check_rep=False), keep_unused=True)
    concat_zeros = [np.zeros((NCORE * z.shape[0], *z.shape[1:]), z.dtype)
                    for z in zero_outs]
    _EXEC = (fn, in_names, out_names, out_avals, concat_zeros)
    return _EXEC


def kernel(query, key, value, w_q, w_k, w_v, w_o):
    query = np.asarray(query, dtype=np.float32)
    key = np.asarray(key, dtype=np.float32)
    value = np.asarray(value, dtype=np.float32)
    ins = make_in_maps(query, key, value, np.asarray(w_q, np.float32),
                       np.asarray(w_k, np.float32), np.asarray(w_v, np.float32),
                       np.asarray(w_o, np.float32))
    fn, in_names, out_names, out_avals, concat_zeros = _get_exec()
    concat_in = [np.concatenate([np.asarray(ins[c][nm]) for c in range(NCORE)])
                 for nm in in_names]
    out_arrs = fn(*concat_in, *concat_zeros)
    results = [
        {nm: np.asarray(out_arrs[i]).reshape(NCORE, *out_avals[i].shape)[c]
         for i, nm in enumerate(out_names)}
        for c in range(NCORE)]
    return assemble(results)


if __name__ == "__main__":
    np.random.seed(0)
    q = np.random.randn(B, Q, HID).astype(np.float32)
    k = np.random.randn(B, KL, HID).astype(np.float32)
    v = np.random.randn(B, KL, HID).astype(np.float32)
    s = 1.0 / np.sqrt(HID)
    wq = (np.random.randn(HID, HID) * s).astype(np.float32)
    wk = (np.random.randn(HID, HID) * s).astype(np.float32)
    wv = (np.random.randn(HID, HID) * s).astype(np.float32)
    wo = (np.random.randn(HID, HID) * s).astype(np.float32)
    t0 = time.time()
    out = kernel(q, k, v, wq, wk, wv, wo)
    print("kernel done", time.time() - t0, out.shape)


# revision 61
# speedup vs baseline: 1.4882x; 1.0614x over previous
"""Trainium2 Bass kernel for MemoryEfficientCrossAttention (seq-parallel v2).

Problem (hardcoded): B=2, Q=2048, K=4096, HIDDEN=1024, HEADS=16, HEAD_DIM=64.
  out = softmax((x_q W_q)(x_k W_k)^T / sqrt(64)) (x_v W_v) W_o

Sharding over 8 NeuronCores: core = b*4 + kq
  b in {0,1}: batch;  kq in {0..3}: K-quarter (1024 contiguous k rows)
Each core projects q for ALL 2048 rows of its batch (4x redundant but cheap),
k/v for its local 1024 k rows (no redundancy), computes unnormalized
attention ctx^T (with a ones-column for the softmax denominators) against its
local k, then a 4-core ReduceScatter per 512-q block sums the partial ctx and
scatters 128-q fragments; each core normalizes its fragments and applies W_o
for its 4x128 output rows.  No other collectives.

Precision: activations and weights are cast to bf16 for the QKV projections
and attention matmuls (psum accumulation stays f32); the xbar DMA-transpose of
x runs on the single bf16 plane.  W_o path stays f32r.  Expected rel err
~5e-3 versus the f32 reference (gate is 2e-2).
"""

import os
import sys
import time

import numpy as np

sys.path.insert(0, "/opt/trn_rl_repo")

import concourse.bass as bass  # noqa: E402
import concourse.mybir as mybir  # noqa: E402
import concourse.tile as tile  # noqa: E402
from concourse import bacc  # noqa: E402

F32 = mybir.dt.float32
F32R = mybir.dt.float32r
BF16 = mybir.dt.bfloat16

HID = 1024
HEADS = 16
HD = 64
B = 2
Q = 2048
KL = 4096
NCORE = 8
KLOC = KL // 4        # local k rows per core
NCH = HID // 128      # 8 hidden chunks
CH_RT = 2             # 128-row tiles per transpose chunk
CHR = CH_RT * 128     # 256 rows per chunk
NKB = KLOC // 128     # 8 local k-blocks
NQB = 4               # 512-q blocks
QB = Q // NQB         # 512
GKB = 2               # k-blocks per score/exp group
SCALE = HD ** -0.5

_CACHED_NC = None


def _build():
    nc = bacc.Bacc("TRN2", target_bir_lowering=False, debug=False,
                   num_devices=NCORE)

    x_q = nc.dram_tensor("x_q", [Q, HID], F32, kind="ExternalInput")
    x_k = nc.dram_tensor("x_k", [KLOC, HID], F32, kind="ExternalInput")
    x_v = nc.dram_tensor("x_v", [KLOC, HID], F32, kind="ExternalInput")
    w_q = nc.dram_tensor("w_q", [HID, HID], F32, kind="ExternalInput")
    w_k = nc.dram_tensor("w_k", [HID, HID], F32, kind="ExternalInput")
    w_v = nc.dram_tensor("w_v", [HID, HID], F32, kind="ExternalInput")
    w_o = nc.dram_tensor("w_o", [HID, HID], F32, kind="ExternalInput")
    out_frag = nc.dram_tensor("out_frag", [NQB, 128, HID], F32,
                              kind="ExternalOutput")

    from contextlib import ExitStack

    with tile.TileContext(nc, pool_alloc_mode="queue") as tc:
        with tc.tile_pool(name="dram", bufs=1, space="DRAM") as dram:
            est = ExitStack()
            pp = est.enter_context(tc.tile_pool(name="persist", bufs=1))
            # partial ctx^T per 512-q block:
            # [head-half, qquad, head%8, d(+denom), 128 q]
            shared = "Shared" if os.environ.get("KSHARED", "0") == "1" \
                else "Local"
            rs_mode = os.environ.get("KRS", "half")
            # seg_h heads per collective segment; each segment is its own
            # contiguous DRAM tensor (BIR requires contiguous collective in).
            seg_h = HEADS if rs_mode in ("full", "a2a") else                 int(os.environ.get("KSEG", "8"))
            nseg = HEADS // seg_h
            ctx_part = [[dram.tile([4, seg_h, HD + 1, 128], BF16,
                                   name=f"ctx_part{i}_{s}")
                         for s in range(nseg)]
                        for i in range(NQB)]
            a2a_out = [dram.tile([4, HEADS, HD + 1, 128], BF16,
                                 name=f"a2a_out{i}")
                       for i in range(NQB)]
            frag = [dram.tile([HEADS, HD + 1, 128], BF16, name=f"frag{i}",
                              addr_space=shared)
                    for i in range(NQB)]

            qTh = pp.tile([128, NCH, Q], BF16)      # strip s = heads 2s,2s+1
            kTh = pp.tile([128, NCH, KLOC], BF16)
            v_aug = pp.tile([128, NKB, HEADS, HD + 1], BF16)

            ones = pp.tile([128, NKB * HEADS], BF16, name="ones")
            nc.vector.memset(ones[:], 1.0)
            nc.vector.tensor_copy(
                v_aug[:, :, :, HD],
                ones[:].rearrange("p (a b) -> p a b", a=NKB))

            # ---------------- staging + projections ----------------
            proj_st = ExitStack()
            xs = proj_st.enter_context(tc.tile_pool(name="xstage", bufs=2))
            xb = proj_st.enter_context(tc.tile_pool(name="xbf", bufs=3))
            xbt = proj_st.enter_context(tc.tile_pool(name="xT", bufs=5))
            ws = proj_st.enter_context(tc.tile_pool(name="wstage", bufs=1))
            wb = proj_st.enter_context(tc.tile_pool(name="wbf", bufs=8))
            pj = proj_st.enter_context(
                tc.tile_pool(name="pproj", bufs=2, space="PSUM"))

            def proj_ps(name):
                t = pj.tile([128, CHR], F32, tag="ps", name=name)
                return t[:]

            def stage_chunk(src, row0, eng=None):
                """bf16-transposed [128, NCH, CHR] chunk of src rows.

                Loads go on SP (or gpsimd for the late wave); transposes go on
                the Activation queue so SP's in-order queue never serializes
                load(n+1) behind transpose(n)'s wait on the DVE cast.
                """
                eng = eng or nc.sync
                xf = xs.tile([128, CH_RT, HID], F32, tag="xf")
                eng.dma_start(
                    xf[:],
                    src[row0:row0 + CHR, :].rearrange("(t p) c -> p t c",
                                                      p=128))
                xc = xb.tile([128, CH_RT, HID], BF16, tag="xc")
                nc.vector.tensor_copy(xc[:], xf[:])
                # single xbar transpose of the flattened [128, 2048] block:
                # out[:, t, hc, r] = rows row0+t*128+r of hidden chunk hc
                xT = xbt.tile([128, CH_RT, NCH, 128], BF16, tag="xT")
                nc.scalar.dma_start_transpose(
                    xT[:].rearrange("p t h r -> p (t h) r"),
                    xc[:].rearrange("p t c -> p (t c)"))
                return xT

            def load_w_bf16(wdram, stage=None):
                """Load+cast w in quarters, optionally interleaving the
                staging of x chunks between quarters so the DMA device
                alternates weight and activation traffic at startup."""
                quarters = []
                for qq in range(4):
                    wf = ws.tile([128, 2, HID], F32, tag="wf")
                    nc.gpsimd.dma_start(
                        wf[:],
                        wdram[qq * CHR:(qq + 1) * CHR, :].rearrange(
                            "(t p) c -> p t c", p=128))
                    wh = wb.tile([128, 2, HID], BF16, tag="wb")
                    nc.scalar.copy(wh[:], wf[:])
                    quarters.append(wh)
                    if stage is not None:
                        stage(qq)
                return quarters

            def wsl(quarters, hc, csl):
                return quarters[hc // 2][:, hc % 2, csl]

            # K projection -> kTh (transposed strips), no redundancy.
            k_xT = []
            wk_h = load_w_bf16(
                w_k, stage=lambda c: k_xT.append(stage_chunk(x_k, c * CHR)))
            for c in range(KLOC // CHR):
                for s in range(NCH):
                    ps = proj_ps(f"psk_{c}_{s}")
                    for hc in range(NCH):
                        nc.tensor.matmul(
                            ps,
                            wsl(wk_h, hc, slice(s * 128, (s + 1) * 128)),
                            k_xT[c][:, :, hc, :],
                            start=(hc == 0), stop=(hc == NCH - 1))
                    nc.vector.tensor_copy(
                        kTh[:, s, c * CHR:(c + 1) * CHR], ps)

            # V projection -> v_aug (natural layout), no redundancy.
            v_xT = []
            wv_h = load_w_bf16(
                w_v, stage=lambda c: v_xT.append(stage_chunk(x_v, c * CHR)))
            for c in range(KLOC // CHR):
                for t in range(CH_RT):
                    kb = c * CH_RT + t
                    for sg in range(4):  # 256-col groups = 4 heads each
                        ps = proj_ps(f"psv_{kb}_{sg}")
                        for hc in range(NCH):
                            nc.tensor.matmul(
                                ps,
                                v_xT[c][:, t, hc, :],
                                wsl(wv_h, hc, slice(sg * 256, (sg + 1) * 256)),
                                start=(hc == 0), stop=(hc == NCH - 1))
                        nc.vector.tensor_copy(
                            v_aug[:, kb, 4 * sg:4 * sg + 4, 0:HD],
                            ps.rearrange("p (h d) -> p h d", h=4))

            # Q staging wave 1 (chunks 0-3, SP queue); wave 2 (4-7) issues on
            # the gpsimd queue mid-attention so its DMAs don't sit behind the
            # attention ctx stores on SP.
            q_xT = []
            wq_h = load_w_bf16(
                w_q, stage=lambda c: q_xT.append(stage_chunk(x_q, c * CHR)))

            def qproj_chunk(c):
                for s in range(NCH):
                    ps = proj_ps(f"psq_{c}_{s}")
                    for hc in range(NCH):
                        nc.tensor.matmul(
                            ps,
                            wsl(wq_h, hc, slice(s * 128, (s + 1) * 128)),
                            q_xT[c][:, :, hc, :],
                            start=(hc == 0), stop=(hc == NCH - 1))
                    nc.vector.tensor_copy(
                        qTh[:, s, c * CHR:(c + 1) * CHR], ps)

            # ---------------- attention + RS + output ----------------
            att_st = ExitStack()
            stp = att_st.enter_context(
                tc.tile_pool(name="pscore", bufs=2, space="PSUM"))
            ctxp = att_st.enter_context(
                tc.tile_pool(name="pctx", bufs=2, space="PSUM"))
            apool = att_st.enter_context(tc.tile_pool(name="apool", bufs=4))
            csb = att_st.enter_context(tc.tile_pool(name="ctxsb", bufs=2))
            wop = att_st.enter_context(tc.tile_pool(name="wo", bufs=1))
            fp = att_st.enter_context(tc.tile_pool(name="fragp", bufs=1))
            rp = att_st.enter_context(tc.tile_pool(name="normp", bufs=1))
            cnp = att_st.enter_context(tc.tile_pool(name="ctxNp", bufs=4))

            def qproj_strip(c, s):
                ps = proj_ps(f"pss_{c}_{s}")
                for hc in range(NCH):
                    nc.tensor.matmul(
                        ps,
                        wsl(wq_h, hc, slice(s * 128, (s + 1) * 128)),
                        q_xT[c][:, :, hc, :],
                        start=(hc == 0), stop=(hc == NCH - 1))
                nc.vector.tensor_copy(
                    qTh[:, s, c * CHR:(c + 1) * CHR], ps)

            def attn_qb(qb, next_chunks=None):
                qsl = slice(qb * QB, (qb + 1) * QB)
                for h in range(HEADS):
                    # Interleave one q-proj strip for the next q-block per
                    # head: keeps Act's exp stream dense across qb boundaries
                    # by filling PE's slack instead of a serial proj block.
                    if next_chunks is not None:
                        nc_c = next_chunks[h // NCH]
                        qproj_strip(nc_c, h % NCH)
                    s, i = h // 2, h % 2
                    psl = slice(i * HD, (i + 1) * HD)
                    ctx = ctxp.tile([HD + 1, QB], F32, tag="ctx",
                                    name=f"ctx_{qb}_{h}")
                    sts = []
                    As = []

                    def scores(g):
                        st = stp.tile([128, GKB, QB], F32, tag="st",
                                      name=f"st_{qb}_{h}_{g}")
                        for j in range(GKB):
                            kb = g * GKB + j
                            nc.tensor.matmul(
                                st[:, j, :],
                                kTh[psl, s, kb * 128:(kb + 1) * 128],
                                qTh[psl, s, qsl],
                                start=True, stop=True)
                        sts.append(st)
                        a = apool.tile([128, GKB, QB], BF16, tag="a",
                                       name=f"a_{qb}_{h}_{g}")
                        nc.scalar.activation(
                            a[:], st[:], mybir.ActivationFunctionType.Exp,
                            scale=SCALE)
                        As.append(a)

                    def pv(g):
                        for j in range(GKB):
                            kb = g * GKB + j
                            nc.tensor.matmul(
                                ctx[:],
                                v_aug[:, kb, h, :],
                                As[g][:, j, :],
                                start=(kb == 0), stop=(kb == NKB - 1))

                    ngr = NKB // GKB
                    scores(0)
                    scores(1)
                    for g in range(ngr):
                        if g + 2 < ngr:
                            scores(g + 2)
                        pv(g)
                    ctx_sb = csb.tile([HD + 1, QB], BF16, tag="ctx_sb",
                                      name=f"ctxsb_{qb}_{h}")
                    nc.vector.tensor_copy(ctx_sb[:], ctx[:])
                    nc.sync.dma_start(
                        ctx_part[qb][h // seg_h][:, h % seg_h, :, :].rearrange(
                            "a p b -> p a b"),
                        ctx_sb[:].rearrange("p (a b) -> p a b", a=4))
                    if (h + 1) % seg_h == 0:
                        rs_qb(qb, h // seg_h)

            RGROUPS = [[0, 1, 2, 3], [4, 5, 6, 7]]

            def rs_qb(qb, seg):
                if rs_mode == "none":
                    return  # timing-only floor: skip collectives
                if rs_mode == "a2a":
                    nc.gpsimd.collective_compute(
                        "AllToAll", mybir.AluOpType.bypass,
                        ins=[ctx_part[qb][seg][:]],
                        outs=[a2a_out[qb][:]],
                        replica_groups=RGROUPS)
                    return
                nc.gpsimd.collective_compute(
                    "ReduceScatter", mybir.AluOpType.add,
                    ins=[ctx_part[qb][seg][:]],
                    outs=[frag[qb][seg * seg_h:(seg + 1) * seg_h]],
                    replica_groups=RGROUPS)

            state = {}

            def load_wo():
                wo_sb = wop.tile([128, NCH, HID], BF16)
                for qq in range(4):
                    wf = ws.tile([128, 2, HID], F32, tag="wf")
                    for j in range(2):
                        hc = qq * 2 + j
                        nc.gpsimd.dma_start(wf[:, j, :],
                                            w_o[hc * 128:(hc + 1) * 128, :])
                    nc.vector.tensor_copy(
                        wo_sb[:, qq * 2:qq * 2 + 2, :], wf[:])
                state["wo_sb"] = wo_sb

            ctxNs = {}

            def norm_qb(qb):
                fsb = fp.tile([HD + 1, HEADS, 128], BF16, tag="fsb")
                if rs_mode == "a2a":
                    # sum the 4 source blocks routed to us by the AllToAll
                    fsc = fp.tile([HD + 1, HEADS, 128], BF16, tag="fsc")
                    nc.gpsimd.dma_start(
                        fsb[:], a2a_out[qb][0].rearrange("h p b -> p h b"))
                    for src in range(1, 4):
                        nc.gpsimd.dma_start(
                            fsc[:],
                            a2a_out[qb][src].rearrange("h p b -> p h b"))
                        nc.vector.tensor_add(fsb[:], fsb[:], fsc[:])
                else:
                    nc.gpsimd.dma_start(
                        fsb[:], frag[qb][:].rearrange("h p b -> p h b"))
                ctxN = cnp.tile([128, NCH, 128], BF16, tag="ctxN",
                                name=f"ctxN_{qb}")
                for hf in range(2):  # 8-head halves
                    hsl = slice(hf * 8, (hf + 1) * 8)
                    rinv = rp.tile([1, 8, 128], F32, tag="rinv",
                                   name=f"rinv_{qb}_{hf}")
                    nc.vector.reciprocal(rinv[:], fsb[HD:HD + 1, hsl, :])
                    rb = rp.tile([HD, 8, 128], F32, tag="rb",
                                 name=f"rb_{qb}_{hf}")
                    nc.gpsimd.partition_broadcast(rb[:], rinv[:])
                    fsb_r = fsb[0:HD, hsl, :].rearrange(
                        "p (c i) b -> p c i b", i=2)
                    rb_r = rb[:].rearrange("p (c i) b -> p c i b", i=2)
                    for i in range(2):
                        nc.vector.tensor_mul(
                            ctxN[i * HD:(i + 1) * HD, hf * 4:(hf + 1) * 4, :],
                            fsb_r[:, :, i, :],
                            rb_r[:, :, i, :])
                ctxNs[qb] = ctxN

            def out_qb(qb):
                wo_sb = state["wo_sb"]
                ctxN = ctxNs[qb]
                po_sb = fp.tile([128, HID], F32, tag="po_sb")
                po = stp.tile([128, GKB, QB], F32, tag="st",
                              name=f"po_{qb}")
                for half in range(2):
                    osl = slice(half * QB, (half + 1) * QB)
                    for hc in range(NCH):
                        nc.tensor.matmul(po[:, half, :], ctxN[:, hc, :],
                                         wo_sb[:, hc, osl],
                                         start=(hc == 0), stop=(hc == NCH - 1))
                nc.vector.tensor_copy(
                    po_sb[:], po[:].rearrange("p a b -> p (a b)"))
                nc.sync.dma_start(out_frag[qb], po_sb[:])

            # schedule: q-proj strips for block qb+1 are interleaved inside
            # attention block qb (one strip per head); RS issues per head-half.
            qproj_chunk(0)
            qproj_chunk(1)
            attn_qb(0, next_chunks=(2, 3))
            for c in range(4, 8):
                q_xT.append(stage_chunk(x_q, c * CHR, eng=nc.gpsimd))
            load_wo()
            attn_qb(1, next_chunks=(4, 5))
            norm_qb(0)
            attn_qb(2, next_chunks=(6, 7))
            norm_qb(1)
            attn_qb(3)
            norm_qb(2)
            out_qb(0)
            out_qb(1)
            out_qb(2)
            norm_qb(3)
            out_qb(3)
            att_st.close()
            proj_st.close()
            est.close()

    nc.compile()
    return nc


def _get_nc():
    global _CACHED_NC
    if _CACHED_NC is None:
        _CACHED_NC = _build()
    return _CACHED_NC


def make_in_maps(query, key, value, w_q, w_k, w_v, w_o):
    ins = []
    for core in range(NCORE):
        b, kq = core // 4, core % 4
        ins.append({
            "x_q": np.ascontiguousarray(query[b]),
            "x_k": np.ascontiguousarray(key[b][kq * KLOC:(kq + 1) * KLOC]),
            "x_v": np.ascontiguousarray(value[b][kq * KLOC:(kq + 1) * KLOC]),
            "w_q": np.ascontiguousarray(w_q),
            "w_k": np.ascontiguousarray(w_k),
            "w_v": np.ascontiguousarray(w_v),
            "w_o": np.ascontiguousarray(w_o),
        })
    return ins


def assemble(results):
    out = np.empty((B, Q, HID), np.float32)
    for core in range(NCORE):
        b, kq = core // 4, core % 4
        fragr = results[core]["out_frag"]
        for qb in range(NQB):
            r0 = qb * QB + kq * 128
            out[b, r0:r0 + 128, :] = fragr[qb]
    return out


_EXEC = None


def _get_exec():
    """Build the 8-core shard_map executable once; reuse across calls."""
    global _EXEC
    if _EXEC is not None:
        return _EXEC
    import jax
    from jax.sharding import Mesh, PartitionSpec
    from jax.experimental.shard_map import shard_map
    from concourse.bass2jax import (_bass_exec_p, install_neuronx_cc_hook,
                                    partition_id_tensor)

    install_neuronx_cc_hook()
    nc = _get_nc()
    in_names, out_names, out_avals, zero_outs = [], [], [], []
    for alloc in nc.m.functions[0].allocations:
        if not isinstance(alloc, mybir.MemoryLocationSet):
            continue
        name = alloc.memorylocations[0].name
        if alloc.kind == "ExternalInput":
            if name != "partition_id":
                in_names.append(name)
        elif alloc.kind == "ExternalOutput":
            out_names.append(name)
            shape = tuple(alloc.tensor_shape)
            dtype = mybir.dt.np(alloc.dtype)
            out_avals.append(jax.core.ShapedArray(shape, dtype))
            zero_outs.append(np.zeros(shape, dtype))
    partition_name = (nc.partition_id_tensor.name
                      if nc.partition_id_tensor else None)
    all_in = list(in_names) + list(out_names)
    if partition_name:
        all_in.append(partition_name)

    def _body(*args):
        operands = list(args)
        if partition_name is not None:
            operands.append(partition_id_tensor())
        return tuple(_bass_exec_p.bind(
            *operands, out_avals=tuple(out_avals), in_names=tuple(all_in),
            out_names=tuple(out_names), lowering_input_output_aliases=(),
            sim_require_finite=True, sim_require_nnan=True, nc=nc))

    devices = jax.devices()[:NCORE]
    mesh = Mesh(np.asarray(devices), ("core",))
    n_all = len(in_names) + len(out_names)
    fn = jax.jit(shard_map(_body, mesh=mesh,
                           in_specs=(PartitionSpec("core"),) * n_all,
                           out_specs=(PartitionSpec("core"),) * len(out_names),
                           check_rep=False), keep_unused=True)
    concat_zeros = [np.zeros((NCORE * z.shape[0], *z.shape[1:]), z.dtype)
                    for z in zero_outs]
    _EXEC = (fn, in_names, out_names, out_avals, concat_zeros)
    return _EXEC


def kernel(query, key, value, w_q, w_k, w_v, w_o):
    query = np.asarray(query, dtype=np.float32)
    key = np.asarray(key, dtype=np.float32)
    value = np.asarray(value, dtype=np.float32)
    ins = make_in_maps(query, key, value, np.asarray(w_q, np.float32),
                       np.asarray(w_k, np.float32), np.asarray(w_v, np.float32),
                       np.asarray(w_o, np.float32))
    fn, in_names, out_names, out_avals, concat_zeros = _get_exec()
    concat_in = [np.concatenate([np.asarray(ins[c][nm]) for c in range(NCORE)])
                 for nm in in_names]
    out_arrs = fn(*concat_in, *concat_zeros)
    results = [
        {nm: np.asarray(out_arrs[i]).reshape(NCORE, *out_avals[i].shape)[c]
         for i, nm in enumerate(out_names)}
        for c in range(NCORE)]
    return assemble(results)


if __name__ == "__main__":
    np.random.seed(0)
    q = np.random.randn(B, Q, HID).astype(np.float32)
    k = np.random.randn(B, KL, HID).astype(np.float32)
    v = np.random.randn(B, KL, HID).astype(np.float32)
    s = 1.0 / np.sqrt(HID)
    wq = (np.random.randn(HID, HID) * s).astype(np.float32)
    wk = (np.random.randn(HID, HID) * s).astype(np.float32)
    wv = (np.random.randn(HID, HID) * s).astype(np.float32)
    wo = (np.random.randn(HID, HID) * s).astype(np.float32)
    t0 = time.time()
    out = kernel(q, k, v, wq, wk, wv, wo)
    print("kernel done", time.time() - t0, out.shape)
